# revision 4
# baseline (speedup 1.0000x reference)
"""CrossInvolution kernel for 8 Trainium2 NeuronCores.

Math (per batch b):
  t      = relu(bn(w1 @ guide))                       # [RED=64, H*W]
  weight = w2 @ t + b2                                # [G*K*K=784, H*W]
  out[c,p] = sum_k weight[g(c)*49+k, p] * x[c, p+dk] + x[c, p]

Sharding: 8 cores = 2 batches x 4 row-slices of 16 image rows each.
Each core computes its full pipeline on its slice (halo rows of the
feature map come in via host-side padding), so there is no duplicated
compute and no cross-core communication.

Device layout per core:
  - channels on partitions (2 halves of 128), pixels (16x64) on free dim
  - per-tap weights are produced already broadcast to all 128 channel
    lanes by a "replicated matmul2": lhsT holds w2 columns replicated
    16x across each group's channels, rhs is t.  The +b2 bias is folded
    into the PSUM->SBUF copy (ACT bias, per-partition).
  - involution taps: DVE/GPSIMD multiply the fp16 weights by the
    shifted x window.  Two tap columns are accumulated on the PE
    (identity-matmul accumulate into a persistent PSUM tile); the
    other five use a DVE/GPSIMD add tree whose result, plus the
    residual x window, is folded into the same PSUM via identity
    matmuls.  The finished PSUM tile is copied to SBUF and DMA'd out.
"""

import numpy as np

import concourse.bass as bass
import concourse.bacc as bacc
import concourse.mybir as mybir
import concourse.tile as tile
from concourse.bass_utils import run_bass_kernel_spmd

FP = mybir.dt.float32
HP = mybir.dt.float16
N_CORES = 8
C = 256
RED = 64
G = 16
GC = 16
KS = 7
KK = KS * KS  # 49
H = W = 64
ROWS = 16          # image rows per core
PIX = ROWS * W     # 1024 pixels per core
PROWS = ROWS + 6   # padded rows (halo 3 each side)
PW = W + 6         # padded width

RSPLIT = 14        # rows on DVE; rest on GPSIMD
GR = ROWS - RSPLIT

# slots 0..3: kw = 2s (even, xh source); slots 4..6: kw = 2(s-4)+1 (xo)
# E-slots 0,1 are PE-accumulated; slots 2,3 (even) and 4..6 (odd) go
# through the acc7 add tree as tree-slots 3,4 and 0,1,2 respectively.
PE_SLOTS = (0, 1)

TRACE = False
LAST_RESULTS = None

_CACHED_NC = None


def _build_nc():
    nc = bacc.Bacc(
        "TRN2",
        debug=False,
        target_bir_lowering=False,
        num_devices=N_CORES,
    )

    guide = nc.dram_tensor("guide", (C, ROWS, W), HP, kind="ExternalInput")
    feat = nc.dram_tensor("feat", (C, PROWS, PW), FP, kind="ExternalInput")
    w1t = nc.dram_tensor("w1t", (C, RED), HP, kind="ExternalInput")
    w2bc = nc.dram_tensor("w2bc", (2, RED, KK * 128), HP, kind="ExternalInput")
    b2bc = nc.dram_tensor("b2bc", (2, 128, KK), FP, kind="ExternalInput")
    scl = nc.dram_tensor("scl", (RED, 1), FP, kind="ExternalInput")
    shf = nc.dram_tensor("shf", (RED, 1), FP, kind="ExternalInput")
    out = nc.dram_tensor("out", (C, ROWS, W), FP, kind="ExternalOutput")

    ident_h = nc.inline_tensor(np.eye(128, dtype=np.float16), name="identh")
    ident_f = nc.inline_tensor(np.eye(128, dtype=np.float32), name="identf")

    with tile.TileContext(nc) as tc:
        with (
            tc.tile_pool(name="consts", bufs=1) as consts,
            tc.tile_pool(name="big", bufs=1) as big,
            tc.tile_pool(name="work", bufs=3) as work,
            tc.tile_pool(name="psum", bufs=3, space="PSUM") as psum,
            tc.tile_pool(name="opsum", bufs=1, space="PSUM") as opsum,
        ):
            # ---- ACT table preload: tiny dep-free op at t=0 ----
            warm = consts.tile([1, 1], FP)
            nc.vector.memset(warm, 0.0)
            nc.scalar.activation(warm, warm,
                                 mybir.ActivationFunctionType.Relu)

            # ---- loads (order matters for pipeline head) ----
            scl_sb = consts.tile([RED, 1], FP)
            nc.sync.dma_start(scl_sb, scl.ap())
            shf_sb = consts.tile([RED, 1], FP)
            nc.sync.dma_start(shf_sb, shf.ap())
            w1t_sb = [consts.tile([128, RED], HP, tag=f"w1t{i}", name=f"w1t{i}") for i in range(2)]
            guide_sb = [[big.tile([128, ROWS // 2, W], HP, tag=f"gd{i}_{h}",
                                  name=f"gd{i}_{h}") for h in range(2)]
                        for i in range(2)]
            xp_sb = [big.tile([128, PROWS, PW], FP, tag=f"xp{i}", name=f"xp{i}") for i in range(2)]
            for h in range(2):
                for i in range(2):
                    csl = slice(i * 128, (i + 1) * 128)
                    nc.sync.dma_start(
                        guide_sb[i][h],
                        guide.ap()[csl, h * (ROWS // 2):(h + 1) * (ROWS // 2)])
            for i in range(2):
                nc.sync.dma_start(w1t_sb[i], w1t.ap()[i * 128:(i + 1) * 128])
            for i in range(2):
                csl = slice(i * 128, (i + 1) * 128)
                nc.sync.dma_start(xp_sb[i], feat.ap()[csl])
            w2bc_sb = [big.tile([RED, KK * 128], HP, tag=f"w2bc{i}",
                                name=f"w2bc{i}") for i in range(2)]
            b2bc_sb = [consts.tile([128, KK], FP, tag=f"b2bc{i}",
                                   name=f"b2bc{i}") for i in range(2)]
            for i in range(2):
                nc.sync.dma_start(w2bc_sb[i], w2bc.ap()[i])
                nc.sync.dma_start(b2bc_sb[i], b2bc.ap()[i])
            id_sb = consts.tile([128, 128], HP, tag="idh", name="idh")
            nc.sync.dma_start(id_sb, ident_h.ap())
            idf_sb = consts.tile([128, 128], FP, tag="idf", name="idf")
            nc.sync.dma_start(idf_sb, ident_f.ap())

            # ---- matmul1 + BN/relu -> t (fp16) ----
            t_sb = big.tile([RED, PIX], HP)
            for nh in range(2):
                nsl = slice(nh * 512, (nh + 1) * 512)
                t_ps = psum.tile([RED, 512], FP, tag="wb", padded_shape=[RED, 1024])
                for i in range(2):
                    nc.tensor.matmul(
                        t_ps,
                        w1t_sb[i],
                        guide_sb[i][nh].rearrange("p a b -> p (a b)"),
                        start=(i == 0),
                        stop=(i == 1),
                    )
                nc.scalar.activation(
                    t_sb[:, nsl], t_ps,
                    mybir.ActivationFunctionType.Relu,
                    bias=shf_sb[:, :], scale=scl_sb[:, :],
                )

            # fp16 feature copies (even/odd alignment variants)
            xh_sb = [big.tile([128, PROWS * PW], HP, tag=f"xh{i}", name=f"xh{i}")
                     for i in range(2)]
            xo_sb = [big.tile([128, PROWS * PW], HP, tag=f"xo{i}", name=f"xo{i}")
                     for i in range(2)]
            xp_flat = [xp_sb[i].rearrange("p a b -> p (a b)") for i in range(2)]
            for i in range(2):
                nc.vector.tensor_copy(xh_sb[i], xp_flat[i])
                nc.vector.tensor_copy(
                    xo_sb[i][:, 0:PROWS * PW - 1], xp_flat[i][:, 1:PROWS * PW])

            # ---- involution ----
            for i in range(2):
                out_ps = opsum.tile([128, PIX], FP, tag="ops",
                                    padded_shape=[128, 1024], name=f"ops{i}")
                acc7 = work.tile([128, 5 * PIX], HP, tag=f"acc7_{i}",
                                 name=f"acc7_{i}", bufs=1)
                acc7g = work.tile([128, 5, GR, W], HP, tag=f"acc7g_{i}",
                                  name=f"acc7g_{i}", bufs=1) if GR > 0 else None
                for kh in range(KS):
                    wbhE = work.tile([128, 4 * PIX], HP, tag="wbhE", bufs=2)
                    wbhO = work.tile([128, 3 * PIX], HP, tag="wbhO", bufs=2)
                    for s in range(7):
                        kw = 2 * s if s < 4 else 2 * (s - 4) + 1
                        k = kh * KS + kw
                        wb = psum.tile([128, PIX], FP, tag="wb",
                                       padded_shape=[128, 1024])
                        for nh in range(2):
                            nsl = slice(nh * 512, nh * 512 + 512)
                            nc.tensor.matmul(
                                wb[:, nsl],
                                w2bc_sb[i][:, k * 128:(k + 1) * 128],
                                t_sb[:, nsl],
                                start=True,
                                stop=True,
                            )
                        dst = (wbhE[:, s * PIX:(s + 1) * PIX] if s < 4
                               else wbhO[:, (s - 4) * PIX:(s - 3) * PIX])
                        nc.scalar.activation(
                            dst, wb,
                            mybir.ActivationFunctionType.Identity,
                            bias=b2bc_sb[i][:, k:k + 1], scale=1.0,
                        )
                    # products; tree slots: 0,1,2 = odd kw (xo),
                    # 3,4 = even kw 4,6 (xh slots 2,3)
                    o = kh * PW
                    wbhE4 = wbhE.rearrange("p (a b c) -> p a b c", b=ROWS, c=W)
                    wbhO4 = wbhO.rearrange("p (a b c) -> p a b c", b=ROWS, c=W)
                    prodE = work.tile([128, 2 * PIX], HP, tag="prodE", bufs=3)
                    prodE4 = prodE.rearrange("p (a b c) -> p a b c",
                                             b=ROWS, c=W)
                    prodT = prodTg = None
                    if kh != 0:
                        prodT = work.tile([128, 5 * PIX], HP, tag="prodT",
                                          bufs=2)
                        if GR > 0:
                            prodTg = work.tile([128, 5, GR, W], HP,
                                               tag="prodTg", bufs=2)
                    a4 = acc7.rearrange("p (a b c) -> p a b c", b=ROWS, c=W)
                    p4 = None if prodT is None else prodT.rearrange(
                        "p (a b c) -> p a b c", b=ROWS, c=W)
                    for (eng, r0, rn) in (
                        (nc.vector, 0, RSPLIT),
                        (nc.gpsimd, RSPLIT, ROWS - RSPLIT),
                    ):
                        if rn <= 0:
                            continue
                        # PE-slots 0,1 (kw 0,2): separate product tile
                        xsP = bass.AP(
                            tensor=xh_sb[i].tensor,
                            offset=xh_sb[i].offset + o + r0 * PW,
                            ap=[xh_sb[i].ap[0], [2, 2], [PW, rn], [1, W]],
                        )
                        eng.tensor_tensor(
                            prodE4[:, :, r0:r0 + rn, :], xsP,
                            wbhE4[:, 0:2, r0:r0 + rn, :], mybir.AluOpType.mult)
                        # tree slots: odd kw 1,3,5 then even kw 4,6
                        xsO = bass.AP(
                            tensor=xo_sb[i].tensor,
                            offset=xo_sb[i].offset + o + r0 * PW,
                            ap=[xo_sb[i].ap[0], [2, 3], [PW, rn], [1, W]],
                        )
                        xsE = bass.AP(
                            tensor=xh_sb[i].tensor,
                            offset=xh_sb[i].offset + o + 4 + r0 * PW,
                            ap=[xh_sb[i].ap[0], [2, 2], [PW, rn], [1, W]],
                        )
                        if eng is nc.vector:
                            aO = a4[:, 0:3, r0:r0 + rn, :]
                            aE = a4[:, 3:5, r0:r0 + rn, :]
                            pO = None if p4 is None else \
                                p4[:, 0:3, r0:r0 + rn, :]
                            pE = None if p4 is None else \
                                p4[:, 3:5, r0:r0 + rn, :]
                        else:
                            aO = acc7g[:, 0:3]
                            aE = acc7g[:, 3:5]
                            pO = None if prodTg is None else prodTg[:, 0:3]
                            pE = None if prodTg is None else prodTg[:, 3:5]
                        wO = wbhO4[:, :, r0:r0 + rn, :]
                        wE = wbhE4[:, 2:4, r0:r0 + rn, :]
                        if kh == 0:
                            eng.tensor_tensor(aO, xsO, wO, mybir.AluOpType.mult)
                            eng.tensor_tensor(aE, xsE, wE, mybir.AluOpType.mult)
                        else:
                            eng.tensor_tensor(pO, xsO, wO, mybir.AluOpType.mult)
                            eng.tensor_tensor(pE, xsE, wE, mybir.AluOpType.mult)
                            eng.tensor_tensor(
                                a4[:, :, r0:r0 + rn, :] if eng is nc.vector
                                else acc7g[:, :],
                                a4[:, :, r0:r0 + rn, :] if eng is nc.vector
                                else acc7g[:, :],
                                p4[:, :, r0:r0 + rn, :] if eng is nc.vector
                                else prodTg[:, :],
                                mybir.AluOpType.add)
                    # PE: accumulate the two PE-slot product columns
                    for u in range(2):
                        for nh in range(2):
                            nsl = slice(nh * 512, nh * 512 + 512)
                            nc.tensor.matmul(
                                out_ps[:, nsl],
                                id_sb,
                                prodE[:, u * PIX + nh * 512:
                                      u * PIX + nh * 512 + 512],
                                start=(kh == 0 and u == 0),
                                stop=False,
                                skip_group_check=True,
                            )
                # ---- per-half tail: tree + residual ----
                a4 = acc7.rearrange("p (a b c) -> p a b c", b=ROWS, c=W)
                s45 = work.tile([128, RSPLIT * W], HP, tag="s45")
                s45r = s45.rearrange("p (b c) -> p b c", c=W)
                s03 = work.tile([128, 2, RSPLIT, W], HP, tag="s03")
                nc.vector.tensor_tensor(
                    s03, a4[:, 0:2, 0:RSPLIT], a4[:, 2:4, 0:RSPLIT],
                    mybir.AluOpType.add)
                nc.vector.tensor_tensor(
                    s45r, s03[:, 0], s03[:, 1], mybir.AluOpType.add)
                nc.vector.tensor_tensor(
                    s45r, s45r, a4[:, 4, 0:RSPLIT], mybir.AluOpType.add)
                nc.tensor.matmul(out_ps[:, 0:512], id_sb, s45[:, 0:512],
                                 start=False, stop=False,
                                 skip_group_check=True)
                nc.tensor.matmul(out_ps[:, 512:RSPLIT * W], id_sb,
                                 s45[:, 512:RSPLIT * W],
                                 start=False, stop=False,
                                 skip_group_check=True)
                if GR > 0:
                    sg = work.tile([128, 2, GR, W], HP, tag="sg")
                    nc.gpsimd.tensor_tensor(
                        sg, acc7g[:, 0:2], acc7g[:, 2:4],
                        mybir.AluOpType.add)
                    sg2 = work.tile([128, GR * W], HP, tag="sg2")
                    sg2r = sg2.rearrange("p (b c) -> p b c", c=W)
                    nc.gpsimd.tensor_tensor(
                        sg2r, sg[:, 0], sg[:, 1], mybir.AluOpType.add)
                    nc.gpsimd.tensor_tensor(
                        sg2r, sg2r, acc7g[:, 4], mybir.AluOpType.add)
                    nc.tensor.matmul(out_ps[:, RSPLIT * W:PIX], id_sb, sg2,
                                     start=False, stop=False,
                                     skip_group_check=True)
                # residual: += x window (fp32, via fp32 identity)
                for nh in range(2):
                    xres = bass.AP(
                        tensor=xp_sb[i].tensor,
                        offset=xp_sb[i].offset + (3 + nh * 8) * PW + 3,
                        ap=[xp_sb[i].ap[0], [PW, 8], [1, W]],
                    )
                    nc.tensor.matmul(
                        out_ps[:, nh * 512:nh * 512 + 512],
                        idf_sb, xres,
                        start=False, stop=(nh == 1),
                        skip_group_check=True,
                    )
                out_sb = work.tile([128, PIX], FP, tag="outsb", bufs=2)
                nc.scalar.activation(out_sb, out_ps,
                                     mybir.ActivationFunctionType.Copy)
                nc.sync.dma_start(out.ap()[i * 128:(i + 1) * 128], out_sb)

    nc.compile()
    return nc


def kernel(**inputs):
    global _CACHED_NC, LAST_RESULTS
    feature_map = np.asarray(inputs["feature_map"], np.float32)
    guide_map = np.asarray(inputs["guide_map"], np.float32)
    w1 = np.asarray(inputs["w1"], np.float32)
    bn_gamma = np.asarray(inputs["bn_gamma"], np.float32)
    bn_beta = np.asarray(inputs["bn_beta"], np.float32)
    bn_mean = np.asarray(inputs["bn_mean"], np.float32)
    bn_var = np.asarray(inputs["bn_var"], np.float32)
    w2 = np.asarray(inputs["w2"], np.float32)
    b2 = np.asarray(inputs["b2"], np.float32)

    scale = bn_gamma / np.sqrt(bn_var + 1e-5)
    shift = bn_beta - bn_mean * scale
    w1t = np.ascontiguousarray(w1.T).astype(np.float16)    # [256, 64]
    # replicated-mm2 lhsT: w2bc[i][r, k*128 + c] = w2[g(c,i)*KK + k, r],
    # g(c,i) = c//16 + 8i.  bias likewise per channel lane.
    w2gk = w2.reshape(G, KK, RED)        # [g, k, r]
    b2gk = b2.reshape(G, KK)
    w2bc = np.empty((2, RED, KK, 128), np.float32)
    b2bc = np.empty((2, 128, KK), np.float32)
    for i in range(2):
        for gl in range(8):
            g = gl + 8 * i
            csl = slice(gl * 16, (gl + 1) * 16)
            w2bc[i, :, :, csl] = w2gk[g].T[:, :, None]       # [r, k, 1]
            b2bc[i, csl, :] = b2gk[g][None, :]
    w2bc = np.ascontiguousarray(
        w2bc.reshape(2, RED, KK * 128)).astype(np.float16)
    b2bc = np.ascontiguousarray(b2bc)

    fpad = np.pad(feature_map, ((0, 0), (0, 0), (3, 3), (3, 3)))

    in_maps = []
    for core in range(N_CORES):
        b, q = divmod(core, 4)
        r0 = q * ROWS
        in_maps.append(dict(
            guide=np.ascontiguousarray(
                guide_map[b, :, r0:r0 + ROWS, :]).astype(np.float16),
            feat=np.ascontiguousarray(fpad[b, :, r0:r0 + PROWS, :]),
            w1t=w1t, w2bc=w2bc, b2bc=b2bc,
            scl=scale.reshape(-1, 1), shf=shift.reshape(-1, 1),
        ))

    if _CACHED_NC is None:
        _CACHED_NC = _build_nc()
    nc = _CACHED_NC

    br = run_bass_kernel_spmd(
        nc, in_maps, list(range(N_CORES)), trace=TRACE,
    )
    LAST_RESULTS = br

    out = np.empty((2, C, H, W), np.float32)
    for core in range(N_CORES):
        b, q = divmod(core, 4)
        r0 = q * ROWS
        out[b, :, r0:r0 + ROWS, :] = br.results[core]["out"]
    return out


# revision 7
# speedup vs baseline: 1.0819x; 1.0819x over previous
"""CrossInvolution kernel for 8 Trainium2 NeuronCores.

Math (per batch b):
  t      = relu(bn(w1 @ guide))                       # [RED=64, H*W]
  weight = w2 @ t + b2                                # [G*K*K=784, H*W]
  out[c,p] = sum_k weight[g(c)*49+k, p] * x[c, p+dk] + x[c, p]

Sharding: 8 cores = 2 batches x 4 row-slices of 16 image rows each.
Each core computes its full pipeline on its slice (halo rows of the
feature map come in via host-side padding), so there is no duplicated
compute and no cross-core communication.

Device layout per core:
  - channels on partitions (2 halves of 128), pixels (16x64) on free dim
  - per-tap weights are produced already broadcast to all 128 channel
    lanes by a "replicated matmul2": lhsT holds w2 columns replicated
    16x across each group's channels, rhs is t.  The +b2 bias is folded
    into the PSUM->SBUF copy (ACT bias, per-partition).
  - involution taps: DVE/GPSIMD multiply the fp16 weights by the
    shifted x window.  Two tap columns are accumulated on the PE
    (identity-matmul accumulate into a persistent PSUM tile); the
    other five use a DVE/GPSIMD add tree whose result, plus the
    residual x window, is folded into the same PSUM via identity
    matmuls.  The finished PSUM tile is copied to SBUF and DMA'd out.
"""

import numpy as np

import concourse.bass as bass
import concourse.bacc as bacc
import concourse.mybir as mybir
import concourse.tile as tile
from concourse.bass_utils import run_bass_kernel_spmd

FP = mybir.dt.float32
HP = mybir.dt.float16
N_CORES = 8
C = 256
RED = 64
G = 16
GC = 16
KS = 7
KK = KS * KS  # 49
H = W = 64
ROWS = 16          # image rows per core
PIX = ROWS * W     # 1024 pixels per core
PROWS = ROWS + 6   # padded rows (halo 3 each side)
PW = W + 6         # padded width

RSPLIT = 16        # rows on DVE; rest on GPSIMD
GR = ROWS - RSPLIT

# slots 0..3: kw = 2s (even, xh source); slots 4..6: kw = 2(s-4)+1 (xo)
# E-slots 0,1 are PE-accumulated; slots 2,3 (even) and 4..6 (odd) go
# through the acc7 add tree as tree-slots 3,4 and 0,1,2 respectively.
PE_SLOTS = (0, 1)

TRACE = False
LAST_RESULTS = None

_CACHED_NC = None


def _build_nc():
    nc = bacc.Bacc(
        "TRN2",
        debug=False,
        target_bir_lowering=False,
        num_devices=N_CORES,
    )

    guide = nc.dram_tensor("guide", (C, ROWS, W), HP, kind="ExternalInput")
    feat = nc.dram_tensor("feat", (C, PROWS, PW), FP, kind="ExternalInput")
    w1t = nc.dram_tensor("w1t", (C, RED), HP, kind="ExternalInput")
    w2bc = nc.dram_tensor("w2bc", (2, RED, KK * 128), HP, kind="ExternalInput")
    b2bc = nc.dram_tensor("b2bc", (2, 128, KK), FP, kind="ExternalInput")
    scl = nc.dram_tensor("scl", (RED, 1), FP, kind="ExternalInput")
    shf = nc.dram_tensor("shf", (RED, 1), FP, kind="ExternalInput")
    out = nc.dram_tensor("out", (C, ROWS, W), FP, kind="ExternalOutput")

    ident_h = nc.inline_tensor(np.eye(128, dtype=np.float16), name="identh")
    ident_f = nc.inline_tensor(np.eye(128, dtype=np.float32), name="identf")

    with tile.TileContext(nc) as tc:
        with (
            tc.tile_pool(name="consts", bufs=1) as consts,
            tc.tile_pool(name="big", bufs=1) as big,
            tc.tile_pool(name="work", bufs=3) as work,
            tc.tile_pool(name="psum", bufs=3, space="PSUM") as psum,
            tc.tile_pool(name="opsum", bufs=1, space="PSUM") as opsum,
        ):
            # ---- ACT table preload: tiny dep-free op at t=0 ----
            warm = consts.tile([1, 1], FP)
            nc.vector.memset(warm, 0.0)
            nc.scalar.activation(warm, warm,
                                 mybir.ActivationFunctionType.Relu)

            # ---- loads (order matters for pipeline head) ----
            scl_sb = consts.tile([RED, 1], FP)
            nc.sync.dma_start(scl_sb, scl.ap())
            shf_sb = consts.tile([RED, 1], FP)
            nc.sync.dma_start(shf_sb, shf.ap())
            w1t_sb = [consts.tile([128, RED], HP, tag=f"w1t{i}", name=f"w1t{i}") for i in range(2)]
            guide_sb = [[big.tile([128, ROWS // 2, W], HP, tag=f"gd{i}_{h}",
                                  name=f"gd{i}_{h}") for h in range(2)]
                        for i in range(2)]
            xp_sb = [big.tile([128, PROWS, PW], FP, tag=f"xp{i}", name=f"xp{i}") for i in range(2)]
            for h in range(2):
                for i in range(2):
                    csl = slice(i * 128, (i + 1) * 128)
                    nc.sync.dma_start(
                        guide_sb[i][h],
                        guide.ap()[csl, h * (ROWS // 2):(h + 1) * (ROWS // 2)])
            for i in range(2):
                nc.sync.dma_start(w1t_sb[i], w1t.ap()[i * 128:(i + 1) * 128])
            for i in range(2):
                csl = slice(i * 128, (i + 1) * 128)
                nc.sync.dma_start(xp_sb[i], feat.ap()[csl])
            w2bc_sb = [big.tile([RED, KK * 128], HP, tag=f"w2bc{i}",
                                name=f"w2bc{i}") for i in range(2)]
            b2bc_sb = [consts.tile([128, KK], FP, tag=f"b2bc{i}",
                                   name=f"b2bc{i}") for i in range(2)]
            for i in range(2):
                nc.sync.dma_start(w2bc_sb[i], w2bc.ap()[i])
                nc.sync.dma_start(b2bc_sb[i], b2bc.ap()[i])
            id_sb = consts.tile([128, 128], HP, tag="idh", name="idh")
            nc.sync.dma_start(id_sb, ident_h.ap())
            idf_sb = consts.tile([128, 128], FP, tag="idf", name="idf")
            nc.sync.dma_start(idf_sb, ident_f.ap())

            # ---- matmul1 + BN/relu -> t (fp16) ----
            t_sb = big.tile([RED, PIX], HP)
            for nh in range(2):
                nsl = slice(nh * 512, (nh + 1) * 512)
                t_ps = psum.tile([RED, 512], FP, tag="wb", padded_shape=[RED, 1024])
                for i in range(2):
                    nc.tensor.matmul(
                        t_ps,
                        w1t_sb[i],
                        guide_sb[i][nh].rearrange("p a b -> p (a b)"),
                        start=(i == 0),
                        stop=(i == 1),
                    )
                nc.scalar.activation(
                    t_sb[:, nsl], t_ps,
                    mybir.ActivationFunctionType.Relu,
                    bias=shf_sb[:, :], scale=scl_sb[:, :],
                )

            # fp16 feature copies (even/odd alignment variants)
            xh_sb = [big.tile([128, PROWS * PW], HP, tag=f"xh{i}", name=f"xh{i}")
                     for i in range(2)]
            xo_sb = [big.tile([128, PROWS * PW], HP, tag=f"xo{i}", name=f"xo{i}")
                     for i in range(2)]
            xp_flat = [xp_sb[i].rearrange("p a b -> p (a b)") for i in range(2)]
            for i in range(2):
                nc.vector.tensor_copy(xh_sb[i], xp_flat[i])
                nc.vector.tensor_copy(
                    xo_sb[i][:, 0:PROWS * PW - 1], xp_flat[i][:, 1:PROWS * PW])

            # ---- involution ----
            for i in range(2):
                out_ps = opsum.tile([128, PIX], FP, tag="ops",
                                    padded_shape=[128, 1024], name=f"ops{i}")
                acc7 = work.tile([128, 5 * PIX], HP, tag=f"acc7_{i}",
                                 name=f"acc7_{i}", bufs=1)
                acc7g = work.tile([128, 5, GR, W], HP, tag=f"acc7g_{i}",
                                  name=f"acc7g_{i}", bufs=1) if GR > 0 else None
                for kh in range(KS):
                    wbhE = work.tile([128, 4 * PIX], HP, tag="wbhE", bufs=3)
                    wbhO = work.tile([128, 3 * PIX], HP, tag="wbhO", bufs=3)
                    for s in range(7):
                        kw = 2 * s if s < 4 else 2 * (s - 4) + 1
                        k = kh * KS + kw
                        wb = psum.tile([128, PIX], FP, tag="wb",
                                       padded_shape=[128, 1024])
                        for nh in range(2):
                            nsl = slice(nh * 512, nh * 512 + 512)
                            nc.tensor.matmul(
                                wb[:, nsl],
                                w2bc_sb[i][:, k * 128:(k + 1) * 128],
                                t_sb[:, nsl],
                                start=True,
                                stop=True,
                            )
                        dst = (wbhE[:, s * PIX:(s + 1) * PIX] if s < 4
                               else wbhO[:, (s - 4) * PIX:(s - 3) * PIX])
                        nc.scalar.activation(
                            dst, wb,
                            mybir.ActivationFunctionType.Identity,
                            bias=b2bc_sb[i][:, k:k + 1], scale=1.0,
                        )
                    # products; tree slots: 0,1,2 = odd kw (xo),
                    # 3,4 = even kw 4,6 (xh slots 2,3)
                    o = kh * PW
                    wbhE4 = wbhE.rearrange("p (a b c) -> p a b c", b=ROWS, c=W)
                    wbhO4 = wbhO.rearrange("p (a b c) -> p a b c", b=ROWS, c=W)
                    prodE = work.tile([128, 2 * PIX], HP, tag="prodE", bufs=4)
                    prodE4 = prodE.rearrange("p (a b c) -> p a b c",
                                             b=ROWS, c=W)
                    prodT = prodTg = None
                    if kh != 0:
                        prodT = work.tile([128, 5 * PIX], HP, tag="prodT",
                                          bufs=2)
                        if GR > 0:
                            prodTg = work.tile([128, 5, GR, W], HP,
                                               tag="prodTg", bufs=2)
                    a4 = acc7.rearrange("p (a b c) -> p a b c", b=ROWS, c=W)
                    p4 = None if prodT is None else prodT.rearrange(
                        "p (a b c) -> p a b c", b=ROWS, c=W)
                    for (eng, r0, rn) in (
                        (nc.vector, 0, RSPLIT),
                        (nc.gpsimd, RSPLIT, ROWS - RSPLIT),
                    ):
                        if rn <= 0:
                            continue
                        # PE-slots 0,1 (kw 0,2): separate product tile
                        xsP = bass.AP(
                            tensor=xh_sb[i].tensor,
                            offset=xh_sb[i].offset + o + r0 * PW,
                            ap=[xh_sb[i].ap[0], [2, 2], [PW, rn], [1, W]],
                        )
                        eng.tensor_tensor(
                            prodE4[:, :, r0:r0 + rn, :], xsP,
                            wbhE4[:, 0:2, r0:r0 + rn, :], mybir.AluOpType.mult)
                        # tree slots: odd kw 1,3,5 then even kw 4,6
                        xsO = bass.AP(
                            tensor=xo_sb[i].tensor,
                            offset=xo_sb[i].offset + o + r0 * PW,
                            ap=[xo_sb[i].ap[0], [2, 3], [PW, rn], [1, W]],
                        )
                        xsE = bass.AP(
                            tensor=xh_sb[i].tensor,
                            offset=xh_sb[i].offset + o + 4 + r0 * PW,
                            ap=[xh_sb[i].ap[0], [2, 2], [PW, rn], [1, W]],
                        )
                        if eng is nc.vector:
                            aO = a4[:, 0:3, r0:r0 + rn, :]
                            aE = a4[:, 3:5, r0:r0 + rn, :]
                            pO = None if p4 is None else \
                                p4[:, 0:3, r0:r0 + rn, :]
                            pE = None if p4 is None else \
                                p4[:, 3:5, r0:r0 + rn, :]
                        else:
                            aO = acc7g[:, 0:3]
                            aE = acc7g[:, 3:5]
                            pO = None if prodTg is None else prodTg[:, 0:3]
                            pE = None if prodTg is None else prodTg[:, 3:5]
                        wO = wbhO4[:, :, r0:r0 + rn, :]
                        wE = wbhE4[:, 2:4, r0:r0 + rn, :]
                        if kh == 0:
                            eng.tensor_tensor(aO, xsO, wO, mybir.AluOpType.mult)
                            eng.tensor_tensor(aE, xsE, wE, mybir.AluOpType.mult)
                        else:
                            eng.tensor_tensor(pO, xsO, wO, mybir.AluOpType.mult)
                            eng.tensor_tensor(pE, xsE, wE, mybir.AluOpType.mult)
                            eng.tensor_tensor(
                                a4[:, :, r0:r0 + rn, :] if eng is nc.vector
                                else acc7g[:, :],
                                a4[:, :, r0:r0 + rn, :] if eng is nc.vector
                                else acc7g[:, :],
                                p4[:, :, r0:r0 + rn, :] if eng is nc.vector
                                else prodTg[:, :],
                                mybir.AluOpType.add)
                    # PE: accumulate the two PE-slot product columns
                    for u in range(2):
                        for nh in range(2):
                            nsl = slice(nh * 512, nh * 512 + 512)
                            nc.tensor.matmul(
                                out_ps[:, nsl],
                                id_sb,
                                prodE[:, u * PIX + nh * 512:
                                      u * PIX + nh * 512 + 512],
                                start=(kh == 0 and u == 0),
                                stop=False,
                                skip_group_check=True,
                            )
                # ---- per-half tail: tree + residual ----
                a4 = acc7.rearrange("p (a b c) -> p a b c", b=ROWS, c=W)
                s45 = work.tile([128, RSPLIT * W], HP, tag="s45")
                s45r = s45.rearrange("p (b c) -> p b c", c=W)
                s03 = work.tile([128, 2, RSPLIT, W], HP, tag="s03")
                nc.vector.tensor_tensor(
                    s03, a4[:, 0:2, 0:RSPLIT], a4[:, 2:4, 0:RSPLIT],
                    mybir.AluOpType.add)
                nc.vector.tensor_tensor(
                    s45r, s03[:, 0], s03[:, 1], mybir.AluOpType.add)
                nc.vector.tensor_tensor(
                    s45r, s45r, a4[:, 4, 0:RSPLIT], mybir.AluOpType.add)
                nc.tensor.matmul(out_ps[:, 0:512], id_sb, s45[:, 0:512],
                                 start=False, stop=False,
                                 skip_group_check=True)
                nc.tensor.matmul(out_ps[:, 512:RSPLIT * W], id_sb,
                                 s45[:, 512:RSPLIT * W],
                                 start=False, stop=False,
                                 skip_group_check=True)
                if GR > 0:
                    sg = work.tile([128, 2, GR, W], HP, tag="sg")
                    nc.gpsimd.tensor_tensor(
                        sg, acc7g[:, 0:2], acc7g[:, 2:4],
                        mybir.AluOpType.add)
                    sg2 = work.tile([128, GR * W], HP, tag="sg2")
                    sg2r = sg2.rearrange("p (b c) -> p b c", c=W)
                    nc.gpsimd.tensor_tensor(
                        sg2r, sg[:, 0], sg[:, 1], mybir.AluOpType.add)
                    nc.gpsimd.tensor_tensor(
                        sg2r, sg2r, acc7g[:, 4], mybir.AluOpType.add)
                    nc.tensor.matmul(out_ps[:, RSPLIT * W:PIX], id_sb, sg2,
                                     start=False, stop=False,
                                     skip_group_check=True)
                # residual: += x window (fp32, via fp32 identity)
                for nh in range(2):
                    xres = bass.AP(
                        tensor=xp_sb[i].tensor,
                        offset=xp_sb[i].offset + (3 + nh * 8) * PW + 3,
                        ap=[xp_sb[i].ap[0], [PW, 8], [1, W]],
                    )
                    nc.tensor.matmul(
                        out_ps[:, nh * 512:nh * 512 + 512],
                        idf_sb, xres,
                        start=False, stop=(nh == 1),
                        skip_group_check=True,
                    )
                out_sb = work.tile([128, PIX], FP, tag="outsb", bufs=2)
                nc.scalar.activation(out_sb, out_ps,
                                     mybir.ActivationFunctionType.Copy)
                nc.sync.dma_start(out.ap()[i * 128:(i + 1) * 128], out_sb)

    nc.compile()
    return nc


def kernel(**inputs):
    global _CACHED_NC, LAST_RESULTS
    feature_map = np.asarray(inputs["feature_map"], np.float32)
    guide_map = np.asarray(inputs["guide_map"], np.float32)
    w1 = np.asarray(inputs["w1"], np.float32)
    bn_gamma = np.asarray(inputs["bn_gamma"], np.float32)
    bn_beta = np.asarray(inputs["bn_beta"], np.float32)
    bn_mean = np.asarray(inputs["bn_mean"], np.float32)
    bn_var = np.asarray(inputs["bn_var"], np.float32)
    w2 = np.asarray(inputs["w2"], np.float32)
    b2 = np.asarray(inputs["b2"], np.float32)

    scale = bn_gamma / np.sqrt(bn_var + 1e-5)
    shift = bn_beta - bn_mean * scale
    w1t = np.ascontiguousarray(w1.T).astype(np.float16)    # [256, 64]
    # replicated-mm2 lhsT: w2bc[i][r, k*128 + c] = w2[g(c,i)*KK + k, r],
    # g(c,i) = c//16 + 8i.  bias likewise per channel lane.
    w2gk = w2.reshape(G, KK, RED)        # [g, k, r]
    b2gk = b2.reshape(G, KK)
    w2bc = np.empty((2, RED, KK, 128), np.float32)
    b2bc = np.empty((2, 128, KK), np.float32)
    for i in range(2):
        for gl in range(8):
            g = gl + 8 * i
            csl = slice(gl * 16, (gl + 1) * 16)
            w2bc[i, :, :, csl] = w2gk[g].T[:, :, None]       # [r, k, 1]
            b2bc[i, csl, :] = b2gk[g][None, :]
    w2bc = np.ascontiguousarray(
        w2bc.reshape(2, RED, KK * 128)).astype(np.float16)
    b2bc = np.ascontiguousarray(b2bc)

    fpad = np.pad(feature_map, ((0, 0), (0, 0), (3, 3), (3, 3)))

    in_maps = []
    for core in range(N_CORES):
        b, q = divmod(core, 4)
        r0 = q * ROWS
        in_maps.append(dict(
            guide=np.ascontiguousarray(
                guide_map[b, :, r0:r0 + ROWS, :]).astype(np.float16),
            feat=np.ascontiguousarray(fpad[b, :, r0:r0 + PROWS, :]),
            w1t=w1t, w2bc=w2bc, b2bc=b2bc,
            scl=scale.reshape(-1, 1), shf=shift.reshape(-1, 1),
        ))

    if _CACHED_NC is None:
        _CACHED_NC = _build_nc()
    nc = _CACHED_NC

    br = run_bass_kernel_spmd(
        nc, in_maps, list(range(N_CORES)), trace=TRACE,
    )
    LAST_RESULTS = br

    out = np.empty((2, C, H, W), np.float32)
    for core in range(N_CORES):
        b, q = divmod(core, 4)
        r0 = q * ROWS
        out[b, :, r0:r0 + ROWS, :] = br.results[core]["out"]
    return out


# revision 8
# speedup vs baseline: 1.1471x; 1.0603x over previous
"""CrossInvolution kernel for 8 Trainium2 NeuronCores.

Math (per batch b):
  t      = relu(bn(w1 @ guide))                       # [RED=64, H*W]
  weight = w2 @ t + b2                                # [G*K*K=784, H*W]
  out[c,p] = sum_k weight[g(c)*49+k, p] * x[c, p+dk] + x[c, p]

Sharding: 8 cores = 2 batches x 4 row-slices of 16 image rows each.
Each core computes its full pipeline on its slice (halo rows of the
feature map come in via host-side padding), so there is no duplicated
compute and no cross-core communication.

Device layout per core:
  - channels on partitions (2 halves of 128), pixels (16x64) on free dim
  - per-tap weights are produced already broadcast to all 128 channel
    lanes by a "replicated matmul2": lhsT holds w2 columns replicated
    16x across each group's channels, rhs is t.  The +b2 bias is folded
    into the PSUM->SBUF copy (ACT bias, per-partition).
  - involution taps: DVE/GPSIMD multiply the fp16 weights by the
    shifted x window.  Two tap columns are accumulated on the PE
    (identity-matmul accumulate into a persistent PSUM tile); the
    other five use a DVE/GPSIMD add tree whose result, plus the
    residual x window, is folded into the same PSUM via identity
    matmuls.  The finished PSUM tile is copied to SBUF and DMA'd out.
"""

import numpy as np

import concourse.bass as bass
import concourse.bacc as bacc
import concourse.mybir as mybir
import concourse.tile as tile
from concourse.bass_utils import run_bass_kernel_spmd

FP = mybir.dt.float32
HP = mybir.dt.float16
N_CORES = 8
C = 256
RED = 64
G = 16
GC = 16
KS = 7
KK = KS * KS  # 49
H = W = 64
ROWS = 16          # image rows per core
PIX = ROWS * W     # 1024 pixels per core
PROWS = ROWS + 6   # padded rows (halo 3 each side)
PW = W + 6         # padded width

RSPLIT = 16        # rows on DVE; rest on GPSIMD
GR = ROWS - RSPLIT

# slots 0..3: kw = 2s (even, xh source); slots 4..6: kw = 2(s-4)+1 (xo)
# E-slots 0,1 are PE-accumulated; slots 2,3 (even) and 4..6 (odd) go
# through the acc7 add tree as tree-slots 3,4 and 0,1,2 respectively.
PE_SLOTS = (0, 1)

TRACE = False
LAST_RESULTS = None

_CACHED_NC = None


def _build_nc():
    nc = bacc.Bacc(
        "TRN2",
        debug=False,
        target_bir_lowering=False,
        num_devices=N_CORES,
    )

    guide = nc.dram_tensor("guide", (C, ROWS, W), HP, kind="ExternalInput")
    feat = nc.dram_tensor("feat", (C, PROWS, PW), HP, kind="ExternalInput")
    w1t = nc.dram_tensor("w1t", (C, RED), HP, kind="ExternalInput")
    w2bc = nc.dram_tensor("w2bc", (2, RED, KK * 128), HP, kind="ExternalInput")
    b2bc = nc.dram_tensor("b2bc", (2, 128, KK), FP, kind="ExternalInput")
    scl = nc.dram_tensor("scl", (RED, 1), FP, kind="ExternalInput")
    shf = nc.dram_tensor("shf", (RED, 1), FP, kind="ExternalInput")
    out = nc.dram_tensor("out", (C, ROWS, W), FP, kind="ExternalOutput")

    ident_h = nc.inline_tensor(np.eye(128, dtype=np.float16), name="identh")

    with tile.TileContext(nc) as tc:
        with (
            tc.tile_pool(name="consts", bufs=1) as consts,
            tc.tile_pool(name="big", bufs=1) as big,
            tc.tile_pool(name="work", bufs=3) as work,
            tc.tile_pool(name="psum", bufs=3, space="PSUM") as psum,
            tc.tile_pool(name="opsum", bufs=1, space="PSUM") as opsum,
        ):
            # ---- ACT table preload: tiny dep-free op at t=0 ----
            warm = consts.tile([1, 1], FP)
            nc.vector.memset(warm, 0.0)
            nc.scalar.activation(warm, warm,
                                 mybir.ActivationFunctionType.Relu)

            # ---- loads (order matters for pipeline head) ----
            scl_sb = consts.tile([RED, 1], FP)
            nc.sync.dma_start(scl_sb, scl.ap())
            shf_sb = consts.tile([RED, 1], FP)
            nc.sync.dma_start(shf_sb, shf.ap())
            w1t_sb = [consts.tile([128, RED], HP, tag=f"w1t{i}", name=f"w1t{i}") for i in range(2)]
            guide_sb = [[big.tile([128, ROWS // 2, W], HP, tag=f"gd{i}_{h}",
                                  name=f"gd{i}_{h}") for h in range(2)]
                        for i in range(2)]
            for h in range(2):
                for i in range(2):
                    csl = slice(i * 128, (i + 1) * 128)
                    nc.sync.dma_start(
                        guide_sb[i][h],
                        guide.ap()[csl, h * (ROWS // 2):(h + 1) * (ROWS // 2)])
            for i in range(2):
                nc.sync.dma_start(w1t_sb[i], w1t.ap()[i * 128:(i + 1) * 128])
            w2bc_sb = [big.tile([RED, KK * 128], HP, tag=f"w2bc{i}",
                                name=f"w2bc{i}") for i in range(2)]
            b2bc_sb = [consts.tile([128, KK], FP, tag=f"b2bc{i}",
                                   name=f"b2bc{i}") for i in range(2)]
            for i in range(2):
                nc.sync.dma_start(w2bc_sb[i], w2bc.ap()[i])
                nc.sync.dma_start(b2bc_sb[i], b2bc.ap()[i])
            id_sb = consts.tile([128, 128], HP, tag="idh", name="idh")
            nc.sync.dma_start(id_sb, ident_h.ap())

            # ---- matmul1 + BN/relu -> t (fp16) ----
            t_sb = big.tile([RED, PIX], HP)
            for nh in range(2):
                nsl = slice(nh * 512, (nh + 1) * 512)
                t_ps = psum.tile([RED, 512], FP, tag="wb", padded_shape=[RED, 1024])
                for i in range(2):
                    nc.tensor.matmul(
                        t_ps,
                        w1t_sb[i],
                        guide_sb[i][nh].rearrange("p a b -> p (a b)"),
                        start=(i == 0),
                        stop=(i == 1),
                    )
                nc.scalar.activation(
                    t_sb[:, nsl], t_ps,
                    mybir.ActivationFunctionType.Relu,
                    bias=shf_sb[:, :], scale=scl_sb[:, :],
                )

            # fp16 feature copies (even/odd alignment variants)
            xh_sb = [big.tile([128, PROWS * PW], HP, tag=f"xh{i}", name=f"xh{i}")
                     for i in range(2)]
            xo_sb = [big.tile([128, PROWS * PW], HP, tag=f"xo{i}", name=f"xo{i}")
                     for i in range(2)]
            for i in range(2):
                csl = slice(i * 128, (i + 1) * 128)
                nc.sync.dma_start(
                    xh_sb[i].rearrange("p (a b) -> p a b", b=PW),
                    feat.ap()[csl])
            for i in range(2):
                nc.vector.tensor_copy(
                    xo_sb[i][:, 0:PROWS * PW - 1], xh_sb[i][:, 1:PROWS * PW])

            # ---- involution ----
            for i in range(2):
                out_ps = opsum.tile([128, PIX], FP, tag="ops",
                                    padded_shape=[128, 1024], name=f"ops{i}")
                acc7 = work.tile([128, 5 * PIX], HP, tag=f"acc7_{i}",
                                 name=f"acc7_{i}", bufs=1)
                acc7g = work.tile([128, 5, GR, W], HP, tag=f"acc7g_{i}",
                                  name=f"acc7g_{i}", bufs=1) if GR > 0 else None
                for kh in range(KS):
                    wbhE = work.tile([128, 4 * PIX], HP, tag="wbhE", bufs=3)
                    wbhO = work.tile([128, 3 * PIX], HP, tag="wbhO", bufs=3)
                    for s in range(7):
                        kw = 2 * s if s < 4 else 2 * (s - 4) + 1
                        k = kh * KS + kw
                        wb = psum.tile([128, PIX], FP, tag="wb",
                                       padded_shape=[128, 1024])
                        for nh in range(2):
                            nsl = slice(nh * 512, nh * 512 + 512)
                            nc.tensor.matmul(
                                wb[:, nsl],
                                w2bc_sb[i][:, k * 128:(k + 1) * 128],
                                t_sb[:, nsl],
                                start=True,
                                stop=True,
                            )
                        dst = (wbhE[:, s * PIX:(s + 1) * PIX] if s < 4
                               else wbhO[:, (s - 4) * PIX:(s - 3) * PIX])
                        nc.scalar.activation(
                            dst, wb,
                            mybir.ActivationFunctionType.Identity,
                            bias=b2bc_sb[i][:, k:k + 1], scale=1.0,
                        )
                    # products; tree slots: 0,1,2 = odd kw (xo),
                    # 3,4 = even kw 4,6 (xh slots 2,3)
                    o = kh * PW
                    wbhE4 = wbhE.rearrange("p (a b c) -> p a b c", b=ROWS, c=W)
                    wbhO4 = wbhO.rearrange("p (a b c) -> p a b c", b=ROWS, c=W)
                    prodE = work.tile([128, 2 * PIX], HP, tag="prodE", bufs=4)
                    prodE4 = prodE.rearrange("p (a b c) -> p a b c",
                                             b=ROWS, c=W)
                    prodT = prodTg = None
                    if kh != 0:
                        prodT = work.tile([128, 5 * PIX], HP, tag="prodT",
                                          bufs=2)
                        if GR > 0:
                            prodTg = work.tile([128, 5, GR, W], HP,
                                               tag="prodTg", bufs=2)
                    a4 = acc7.rearrange("p (a b c) -> p a b c", b=ROWS, c=W)
                    p4 = None if prodT is None else prodT.rearrange(
                        "p (a b c) -> p a b c", b=ROWS, c=W)
                    for (eng, r0, rn) in (
                        (nc.vector, 0, RSPLIT),
                        (nc.gpsimd, RSPLIT, ROWS - RSPLIT),
                    ):
                        if rn <= 0:
                            continue
                        # PE-slots 0,1 (kw 0,2): separate product tile
                        xsP = bass.AP(
                            tensor=xh_sb[i].tensor,
                            offset=xh_sb[i].offset + o + r0 * PW,
                            ap=[xh_sb[i].ap[0], [2, 2], [PW, rn], [1, W]],
                        )
                        eng.tensor_tensor(
                            prodE4[:, :, r0:r0 + rn, :], xsP,
                            wbhE4[:, 0:2, r0:r0 + rn, :], mybir.AluOpType.mult)
                        # tree slots: odd kw 1,3,5 then even kw 4,6
                        xsO = bass.AP(
                            tensor=xo_sb[i].tensor,
                            offset=xo_sb[i].offset + o + r0 * PW,
                            ap=[xo_sb[i].ap[0], [2, 3], [PW, rn], [1, W]],
                        )
                        xsE = bass.AP(
                            tensor=xh_sb[i].tensor,
                            offset=xh_sb[i].offset + o + 4 + r0 * PW,
                            ap=[xh_sb[i].ap[0], [2, 2], [PW, rn], [1, W]],
                        )
                        if eng is nc.vector:
                            aO = a4[:, 0:3, r0:r0 + rn, :]
                            aE = a4[:, 3:5, r0:r0 + rn, :]
                            pO = None if p4 is None else \
                                p4[:, 0:3, r0:r0 + rn, :]
                            pE = None if p4 is None else \
                                p4[:, 3:5, r0:r0 + rn, :]
                        else:
                            aO = acc7g[:, 0:3]
                            aE = acc7g[:, 3:5]
                            pO = None if prodTg is None else prodTg[:, 0:3]
                            pE = None if prodTg is None else prodTg[:, 3:5]
                        wO = wbhO4[:, :, r0:r0 + rn, :]
                        wE = wbhE4[:, 2:4, r0:r0 + rn, :]
                        if kh == 0:
                            eng.tensor_tensor(aO, xsO, wO, mybir.AluOpType.mult)
                            eng.tensor_tensor(aE, xsE, wE, mybir.AluOpType.mult)
                        else:
                            eng.tensor_tensor(pO, xsO, wO, mybir.AluOpType.mult)
                            eng.tensor_tensor(pE, xsE, wE, mybir.AluOpType.mult)
                            eng.tensor_tensor(
                                a4[:, :, r0:r0 + rn, :] if eng is nc.vector
                                else acc7g[:, :],
                                a4[:, :, r0:r0 + rn, :] if eng is nc.vector
                                else acc7g[:, :],
                                p4[:, :, r0:r0 + rn, :] if eng is nc.vector
                                else prodTg[:, :],
                                mybir.AluOpType.add)
                    # PE: accumulate the two PE-slot product columns
                    for u in range(2):
                        for nh in range(2):
                            nsl = slice(nh * 512, nh * 512 + 512)
                            nc.tensor.matmul(
                                out_ps[:, nsl],
                                id_sb,
                                prodE[:, u * PIX + nh * 512:
                                      u * PIX + nh * 512 + 512],
                                start=(kh == 0 and u == 0),
                                stop=False,
                                skip_group_check=True,
                            )
                # ---- per-half tail: tree + residual ----
                a4 = acc7.rearrange("p (a b c) -> p a b c", b=ROWS, c=W)
                s45 = work.tile([128, RSPLIT * W], HP, tag="s45")
                s45r = s45.rearrange("p (b c) -> p b c", c=W)
                s03 = work.tile([128, 2, RSPLIT, W], HP, tag="s03")
                nc.vector.tensor_tensor(
                    s03, a4[:, 0:2, 0:RSPLIT], a4[:, 2:4, 0:RSPLIT],
                    mybir.AluOpType.add)
                nc.vector.tensor_tensor(
                    s45r, s03[:, 0], s03[:, 1], mybir.AluOpType.add)
                nc.vector.tensor_tensor(
                    s45r, s45r, a4[:, 4, 0:RSPLIT], mybir.AluOpType.add)
                nc.tensor.matmul(out_ps[:, 0:512], id_sb, s45[:, 0:512],
                                 start=False, stop=False,
                                 skip_group_check=True)
                nc.tensor.matmul(out_ps[:, 512:RSPLIT * W], id_sb,
                                 s45[:, 512:RSPLIT * W],
                                 start=False, stop=False,
                                 skip_group_check=True)
                if GR > 0:
                    sg = work.tile([128, 2, GR, W], HP, tag="sg")
                    nc.gpsimd.tensor_tensor(
                        sg, acc7g[:, 0:2], acc7g[:, 2:4],
                        mybir.AluOpType.add)
                    sg2 = work.tile([128, GR * W], HP, tag="sg2")
                    sg2r = sg2.rearrange("p (b c) -> p b c", c=W)
                    nc.gpsimd.tensor_tensor(
                        sg2r, sg[:, 0], sg[:, 1], mybir.AluOpType.add)
                    nc.gpsimd.tensor_tensor(
                        sg2r, sg2r, acc7g[:, 4], mybir.AluOpType.add)
                    nc.tensor.matmul(out_ps[:, RSPLIT * W:PIX], id_sb, sg2,
                                     start=False, stop=False,
                                     skip_group_check=True)
                # residual: += x window (fp32, via fp32 identity)
                for nh in range(2):
                    xres = bass.AP(
                        tensor=xh_sb[i].tensor,
                        offset=xh_sb[i].offset + (3 + nh * 8) * PW + 3,
                        ap=[xh_sb[i].ap[0], [PW, 8], [1, W]],
                    )
                    nc.tensor.matmul(
                        out_ps[:, nh * 512:nh * 512 + 512],
                        id_sb, xres,
                        start=False, stop=(nh == 1),
                        skip_group_check=True,
                    )
                out_sb = work.tile([128, PIX], FP, tag="outsb", bufs=2)
                nc.scalar.activation(out_sb, out_ps,
                                     mybir.ActivationFunctionType.Copy)
                nc.sync.dma_start(out.ap()[i * 128:(i + 1) * 128], out_sb)

    nc.compile()
    return nc


def kernel(**inputs):
    global _CACHED_NC, LAST_RESULTS
    feature_map = np.asarray(inputs["feature_map"], np.float32)
    guide_map = np.asarray(inputs["guide_map"], np.float32)
    w1 = np.asarray(inputs["w1"], np.float32)
    bn_gamma = np.asarray(inputs["bn_gamma"], np.float32)
    bn_beta = np.asarray(inputs["bn_beta"], np.float32)
    bn_mean = np.asarray(inputs["bn_mean"], np.float32)
    bn_var = np.asarray(inputs["bn_var"], np.float32)
    w2 = np.asarray(inputs["w2"], np.float32)
    b2 = np.asarray(inputs["b2"], np.float32)

    scale = bn_gamma / np.sqrt(bn_var + 1e-5)
    shift = bn_beta - bn_mean * scale
    w1t = np.ascontiguousarray(w1.T).astype(np.float16)    # [256, 64]
    # replicated-mm2 lhsT: w2bc[i][r, k*128 + c] = w2[g(c,i)*KK + k, r],
    # g(c,i) = c//16 + 8i.  bias likewise per channel lane.
    w2gk = w2.reshape(G, KK, RED)        # [g, k, r]
    b2gk = b2.reshape(G, KK)
    w2bc = np.empty((2, RED, KK, 128), np.float32)
    b2bc = np.empty((2, 128, KK), np.float32)
    for i in range(2):
        for gl in range(8):
            g = gl + 8 * i
            csl = slice(gl * 16, (gl + 1) * 16)
            w2bc[i, :, :, csl] = w2gk[g].T[:, :, None]       # [r, k, 1]
            b2bc[i, csl, :] = b2gk[g][None, :]
    w2bc = np.ascontiguousarray(
        w2bc.reshape(2, RED, KK * 128)).astype(np.float16)
    b2bc = np.ascontiguousarray(b2bc)

    fpad = np.pad(feature_map, ((0, 0), (0, 0), (3, 3), (3, 3)))

    in_maps = []
    for core in range(N_CORES):
        b, q = divmod(core, 4)
        r0 = q * ROWS
        in_maps.append(dict(
            guide=np.ascontiguousarray(
                guide_map[b, :, r0:r0 + ROWS, :]).astype(np.float16),
            feat=np.ascontiguousarray(fpad[b, :, r0:r0 + PROWS, :]).astype(np.float16),
            w1t=w1t, w2bc=w2bc, b2bc=b2bc,
            scl=scale.reshape(-1, 1), shf=shift.reshape(-1, 1),
        ))

    if _CACHED_NC is None:
        _CACHED_NC = _build_nc()
    nc = _CACHED_NC

    br = run_bass_kernel_spmd(
        nc, in_maps, list(range(N_CORES)), trace=TRACE,
    )
    LAST_RESULTS = br

    out = np.empty((2, C, H, W), np.float32)
    for core in range(N_CORES):
        b, q = divmod(core, 4)
        r0 = q * ROWS
        out[b, :, r0:r0 + ROWS, :] = br.results[core]["out"]
    return out
